# revision 8
# baseline (speedup 1.0000x reference)
"""Trainium2 Bass kernel for nn_DN (topk_masking): cosine top-1 winner-take-all.

Math (reference):
    xf    = l2norm(x.reshape(B, -1))            # [B, X]
    w_xy  = l2norm_rows(x2y_w)                  # [Y, X]
    y_pre = (xf @ w_xy.T) * (y_age >= 1)        # [B, Y]
    win   = argmax(y_pre, axis=1)               # [B]
    out   = l2norm_rows(y2z_w)[:, win].T        # [B, Z]

Strategy: the only heavy compute is the [B,X] @ [X,Y] score GEMM (137
GFLOP). The device runs it in fp8(e4m3) with DoubleRow perf mode (2 fp8
weights per PE cell -> 2x bf16 throughput), on row-normalized, age-masked,
scaled weights prepared on host. Y (32768) is sharded 8 ways (4096/core).
Each core reduces its scores to per-(b, 512-group) top-8 value+index
candidates with the DVE max8 unit and DMAs the 2x128KB candidate tensors
out. That is the whole device program: no collectives, no second matmul.

The host resolves the global argmax from the 8x256 candidates per row. fp8
quantization error is bounded (measured max |err| = 2.98e-3 in
x-normalized cosine units on this input distribution; DELTA = 6e-3 gives a
2x cushion). Any row whose top-2 candidate margin is within the 2*DELTA
band is re-scored exactly (fp64) over the in-band candidates; a per-group
tail guard (8th candidate still in band -> full-row exact rescore)
guarantees the true winner can never be silently missed. The output rows
are then gathered from host-normalized y2z_w columns.
"""

from dataclasses import dataclass

import numpy as np
import ml_dtypes

import concourse.bass as bass  # noqa: F401  (kept for parity with tooling)
import concourse.mybir as mybir
import concourse.tile as tile
from concourse import bacc
from concourse.bass_utils import run_bass_kernel_spmd

P = 128
F32 = mybir.dt.float32
U32 = mybir.dt.uint32
FP8 = mybir.dt.float8e4
NP_FP8 = ml_dtypes.float8_e4m3fn

SX = 16.0          # x quantization scale
SW = 256.0         # weight-row quantization scale (post-normalization)
SCALE = SX * SW
# Margin (in x-normalized cosine units) below which the host re-checks a
# row. Measured max device-vs-fp64 score error on this input distribution
# is 2.98e-3; 6e-3 gives a 2x cushion.
DELTA = 6e-3


@dataclass(frozen=True)
class Geom:
    B: int = 512          # batch
    X: int = 4096         # input features
    Y: int = 32768        # y neurons (sharded)
    Z: int = 1000         # output classes
    NC: int = 8           # cores
    GW: int = 512         # y-group width (PSUM bank = 512 fp32)

    @property
    def BT(self): return self.B // P          # b tiles
    @property
    def KT(self): return self.X // P          # contraction k-tiles
    @property
    def KP(self): return self.KT // 2         # k-tile pairs (DoubleRow)
    @property
    def YL(self): return self.Y // self.NC    # y per core
    @property
    def G(self): return self.YL // self.GW    # y groups per core
    @property
    def CAND(self): return self.BT * self.G * 8


FULL = Geom()

TRACE = False          # test harness sets True (needs NTFF hook installed)
TRACE_KWARGS = {}
LAST_RESULTS = None    # BassKernelResults of the last run (for profiling)


# --------------------------------------------------------------------------
# device kernel
# --------------------------------------------------------------------------

def build_nc(g: Geom = FULL) -> bacc.Bacc:
    nc = bacc.Bacc("TRN2", target_bir_lowering=False, debug=False,
                   num_devices=g.NC)

    KGW = g.KT * g.GW
    xq_d = nc.dram_tensor("xq", [P, g.KT * g.B], FP8, kind="ExternalInput")
    wq_d = nc.dram_tensor("wq", [P, g.G * KGW], FP8, kind="ExternalInput")
    candv_d = nc.dram_tensor("candv", [P, g.CAND], F32, kind="ExternalOutput")
    candi_d = nc.dram_tensor("candi", [P, g.CAND], U32, kind="ExternalOutput")

    B8 = g.BT * 8
    with tile.TileContext(nc) as tc:
        NCH = 4 if g.KT % 8 == 0 else 1
        kc = g.KT // NCH
        with (
            tc.tile_pool(name="xt_p", bufs=NCH) as xt_p,
            tc.tile_pool(name="wt_p", bufs=3 * NCH) as wt_p,
            tc.tile_pool(name="cand_p", bufs=1) as cand_p,
            tc.tile_pool(name="s_ps", bufs=8, space="PSUM") as s_ps,
        ):
            # x^T and weight slices live in small [P, kc, ...] tiles so the
            # first matmul chain only waits on ~1MB of DMA, not 4MB
            # (dependencies are tile-granular)
            def issue_wt(gi, s):
                wts = wt_p.tile([P, kc, g.GW], FP8, tag="wt",
                                name=f"wt{gi}_{s}")
                nc.sync.dma_start(
                    out=wts[:],
                    in_=wq_d.ap()[:, gi * KGW + s * kc * g.GW:
                                  gi * KGW + (s + 1) * kc * g.GW]
                        .rearrange("p (k w) -> p k w", k=kc))
                return wts

            xts = []

            def issue_xt(s):
                t = xt_p.tile([P, kc, g.B], FP8, name=f"xt{s}")
                nc.sync.dma_start(
                    out=t[:],
                    in_=xq_d.ap()[:, s * kc * g.B:(s + 1) * kc * g.B]
                        .rearrange("p (k b) -> p k b", k=kc))
                xts.append(t)

            # group 0's weights interleaved with x, k-ascending, so the
            # first matmul's inputs (wt0 chunk 0 + xt chunk 0) land first
            wt0 = []
            for s in range(NCH):
                wt0.append(issue_wt(0, s))
                issue_xt(s)

            candv = cand_p.tile([P, g.CAND], F32)
            candi = cand_p.tile([P, g.CAND], U32)

            for gi in range(g.G):
                if gi == 0:
                    wts = wt0
                else:
                    wts = [issue_wt(gi, s) for s in range(NCH)]

                # bi-outer / j-inner: 16-matmul accumulation chains into a
                # single PSUM bank keep the PE at its 216ns/MM stream rate
                # (interleaving banks per-MM measures ~20% slower)
                for bi in range(g.BT):
                    pst = s_ps.tile([P, g.GW], F32, tag="s",
                                    name=f"s{gi}_{bi}")
                    for j in range(g.KP):
                        jj = 2 * j
                        nc.tensor.matmul(
                            pst[:],
                            xts[jj // kc][:, jj % kc:jj % kc + 2,
                                          bi * P:(bi + 1) * P],
                            wts[jj // kc][:, jj % kc:jj % kc + 2, :],
                            start=(j == 0), stop=(j == g.KP - 1),
                            perf_mode=mybir.MatmulPerfMode.DoubleRow)
                    c0 = gi * B8 + bi * 8
                    nc.vector.max(candv[:, c0:c0 + 8], pst[:])
                    nc.vector.max_index(candi[:, c0:c0 + 8],
                                        candv[:, c0:c0 + 8], pst[:])
                    if gi == g.G - 1:
                        # last group: stream per-bi so only 8 columns of
                        # DMA remain after the final max_index
                        nc.sync.dma_start(out=candv_d.ap()[:, c0:c0 + 8],
                                          in_=candv[:, c0:c0 + 8])
                        nc.sync.dma_start(out=candi_d.ap()[:, c0:c0 + 8],
                                          in_=candi[:, c0:c0 + 8])
                if gi < g.G - 1:
                    # stream this group's candidate slab out immediately
                    nc.sync.dma_start(
                        out=candv_d.ap()[:, gi * B8:(gi + 1) * B8],
                        in_=candv[:, gi * B8:(gi + 1) * B8])
                    nc.sync.dma_start(
                        out=candi_d.ap()[:, gi * B8:(gi + 1) * B8],
                        in_=candi[:, gi * B8:(gi + 1) * B8])

    nc.compile()
    return nc


# --------------------------------------------------------------------------
# host side
# --------------------------------------------------------------------------

def prep_inputs(g: Geom, x, x2y_w, y_age):
    """Normalize + mask + quantize on host; lay out k-major per partition."""
    B, X = g.B, g.X
    xf = np.ascontiguousarray(x.reshape(B, X)).astype(np.float32)
    W = x2y_w
    nrm = np.sqrt(np.einsum("yk,yk->y", W, W))
    mask = (y_age[0] >= 1)
    fac = np.where(mask, SW / np.maximum(nrm, 1e-12), 0.0).astype(np.float32)
    wq8 = (W * fac[:, None]).astype(NP_FP8)               # [Y, X]
    xq8 = (xf * SX).astype(NP_FP8)                        # [B, X]

    # device layouts: [p, k, b] and per-core [p, gi, k, w]
    xq_dev = np.ascontiguousarray(
        xq8.reshape(B, g.KT, P).transpose(2, 1, 0)).reshape(P, g.KT * B)
    wq_dev = np.ascontiguousarray(
        wq8.reshape(g.NC, g.G, g.GW, g.KT, P).transpose(0, 4, 1, 3, 2)
    ).reshape(g.NC, P, g.G * g.KT * g.GW)

    return [{"xq": xq_dev, "wq": wq_dev[c]} for c in range(g.NC)]


def postprocess(g: Geom, results, x, x2y_w, y2z_w, y_age):
    """Resolve global argmax from candidates; exact-rescore margin rows."""
    G8 = g.G * 8
    B = g.B
    NCOL = g.NC * G8
    V = np.empty((B, NCOL), np.float32)
    I = np.empty((B, NCOL), np.int64)
    for c in range(g.NC):
        # device layout: [p, (gi, bi, 8)] with b = bi*128 + p
        cv = np.asarray(results[c]["candv"])                 # [P, CAND]
        ci = np.asarray(results[c]["candi"]).astype(np.int64)
        cv = cv.reshape(P, g.G, g.BT, 8).transpose(2, 0, 1, 3).reshape(B, G8)
        ci = ci.reshape(P, g.G, g.BT, 8).transpose(2, 0, 1, 3).reshape(B, G8)
        V[:, c * G8:(c + 1) * G8] = cv
        I[:, c * G8:(c + 1) * G8] = ci
    base = np.repeat(
        (np.arange(g.NC * g.G, dtype=np.int64) * g.GW), 8)  # [NCOL]
    I += base[None, :]

    xf = x.reshape(B, -1).astype(np.float64)
    xn = np.linalg.norm(xf, axis=1)
    mask = (y_age[0] >= 1)

    def exact_c(b, ys):
        ys = np.asarray(ys, dtype=np.int64)
        Wv = x2y_w[ys, :].astype(np.float64)
        c = (Wv @ xf[b]) / np.linalg.norm(Wv, axis=1) / xn[b]
        return np.where(mask[ys], c, 0.0)

    win = np.empty(B, np.int64)
    tail_rows = []
    n_flagged = n_patched = 0
    max_obs_err = 0.0
    band = 2.0 * DELTA
    for b in range(B):
        vb = V[b] / (SCALE * xn[b])          # x-normalized device scores
        ib = I[b]
        vmax = vb.max()
        win[b] = int(ib[vb == vmax].min())
        in_band = vb >= vmax - band
        if int(in_band.sum()) <= 1:
            continue
        n_flagged += 1
        # guard: a group's 8th (weakest reported) candidate still in band
        # means candidates may be missing -> full exact rescore of the row
        tails = vb.reshape(-1, 8)[:, 7]
        if np.any(tails >= vmax - band):
            tail_rows.append(b)
            continue
        ys, idx = np.unique(ib[in_band], return_index=True)
        ce = exact_c(b, ys)
        max_obs_err = max(max_obs_err,
                          float(np.abs(vb[in_band][idx] - ce).max()))
        w_true = int(ys[np.argmax(ce)])
        if w_true != win[b]:
            n_patched += 1
        win[b] = w_true

    if tail_rows:
        T = len(tail_rows)
        S = np.empty((T, g.Y), np.float64)
        xb = xf[tail_rows]
        CH = 4096
        for y0 in range(0, g.Y, CH):
            Wc = x2y_w[y0:y0 + CH].astype(np.float64)
            S[:, y0:y0 + CH] = (xb @ Wc.T) / \
                np.linalg.norm(Wc, axis=1)[None, :]
        S[:, ~mask] = 0.0
        for t, b in enumerate(tail_rows):
            w_true = int(np.argmax(S[t]))
            if w_true != win[b]:
                n_patched += 1
            win[b] = w_true

    n2 = np.sqrt(np.einsum("zy,zy->z", y2z_w, y2z_w, dtype=np.float64))
    n2 = np.maximum(n2, 1e-12)
    out = (y2z_w[:, win].astype(np.float64) / n2[:, None]).T
    postprocess.stats = {"flagged": n_flagged, "patched": n_patched,
                         "full_rescore": len(tail_rows),
                         "max_obs_err": max_obs_err}
    return np.ascontiguousarray(out.astype(np.float32))


_BUILT = {}


def _get_nc(g: Geom):
    if g not in _BUILT:
        _BUILT[g] = build_nc(g)
    return _BUILT[g]


def kernel(**inputs) -> np.ndarray:
    global LAST_RESULTS
    g = FULL
    x = np.asarray(inputs["x"], dtype=np.float32)
    x2y_w = np.asarray(inputs["x2y_w"], dtype=np.float32)
    y2z_w = np.asarray(inputs["y2z_w"], dtype=np.float32)
    y_age = np.asarray(inputs["y_age"])

    nc = _get_nc(g)
    in_maps = prep_inputs(g, x, x2y_w, y_age)
    res = run_bass_kernel_spmd(nc, in_maps, list(range(g.NC)),
                               trace=TRACE, **TRACE_KWARGS)
    LAST_RESULTS = res
    return postprocess(g, res.results, x, x2y_w, y2z_w, y_age)


# revision 9
# speedup vs baseline: 1.0196x; 1.0196x over previous
"""Trainium2 Bass kernel for nn_DN (topk_masking): cosine top-1 winner-take-all.

Math (reference):
    xf    = l2norm(x.reshape(B, -1))            # [B, X]
    w_xy  = l2norm_rows(x2y_w)                  # [Y, X]
    y_pre = (xf @ w_xy.T) * (y_age >= 1)        # [B, Y]
    win   = argmax(y_pre, axis=1)               # [B]
    out   = l2norm_rows(y2z_w)[:, win].T        # [B, Z]

Strategy: the only heavy compute is the [B,X] @ [X,Y] score GEMM (137
GFLOP). The device runs it in fp8(e4m3) with DoubleRow perf mode (2 fp8
weights per PE cell -> 2x bf16 throughput), on row-normalized, age-masked,
scaled weights prepared on host. Y (32768) is sharded 8 ways (4096/core).
Each core reduces its scores to per-(b, 512-group) top-8 value+index
candidates with the DVE max8 unit and DMAs the 2x128KB candidate tensors
out. That is the whole device program: no collectives, no second matmul.

The host resolves the global argmax from the 8x256 candidates per row. fp8
quantization error is bounded (measured max |err| = 2.98e-3 in
x-normalized cosine units on this input distribution; DELTA = 6e-3 gives a
2x cushion). Any row whose top-2 candidate margin is within the 2*DELTA
band is re-scored exactly (fp64) over the in-band candidates; a per-group
tail guard (8th candidate still in band -> full-row exact rescore)
guarantees the true winner can never be silently missed. The output rows
are then gathered from host-normalized y2z_w columns.
"""

from dataclasses import dataclass

import numpy as np
import ml_dtypes

import concourse.bass as bass  # noqa: F401  (kept for parity with tooling)
import concourse.mybir as mybir
import concourse.tile as tile
from concourse import bacc
from concourse.bass_utils import run_bass_kernel_spmd

P = 128
F32 = mybir.dt.float32
U32 = mybir.dt.uint32
FP8 = mybir.dt.float8e4
NP_FP8 = ml_dtypes.float8_e4m3fn

SX = 16.0          # x quantization scale
SW = 256.0         # weight-row quantization scale (post-normalization)
SCALE = SX * SW
# Margin (in x-normalized cosine units) below which the host re-checks a
# row. Measured max device-vs-fp64 score error on this input distribution
# is 2.98e-3; 6e-3 gives a 2x cushion.
DELTA = 6e-3


@dataclass(frozen=True)
class Geom:
    B: int = 512          # batch
    X: int = 4096         # input features
    Y: int = 32768        # y neurons (sharded)
    Z: int = 1000         # output classes
    NC: int = 8           # cores
    GW: int = 512         # y-group width (PSUM bank = 512 fp32)

    @property
    def BT(self): return self.B // P          # b tiles
    @property
    def KT(self): return self.X // P          # contraction k-tiles
    @property
    def KP(self): return self.KT // 2         # k-tile pairs (DoubleRow)
    @property
    def YL(self): return self.Y // self.NC    # y per core
    @property
    def G(self): return self.YL // self.GW    # y groups per core
    @property
    def CAND(self): return self.BT * self.G * 8


FULL = Geom()

TRACE = False          # test harness sets True (needs NTFF hook installed)
TRACE_KWARGS = {}
LAST_RESULTS = None    # BassKernelResults of the last run (for profiling)


# --------------------------------------------------------------------------
# device kernel
# --------------------------------------------------------------------------

def build_nc(g: Geom = FULL) -> bacc.Bacc:
    nc = bacc.Bacc("TRN2", target_bir_lowering=False, debug=False,
                   num_devices=g.NC)

    KGW = g.KT * g.GW
    xq_d = nc.dram_tensor("xq", [P, g.KT * g.B], FP8, kind="ExternalInput")
    wq_d = nc.dram_tensor("wq", [P, g.G * KGW], FP8, kind="ExternalInput")
    candv_d = nc.dram_tensor("candv", [P, g.CAND], F32, kind="ExternalOutput")
    candi_d = nc.dram_tensor("candi", [P, g.CAND], U32, kind="ExternalOutput")

    B8 = g.BT * 8
    with tile.TileContext(nc) as tc:
        kc = g.KT // 4 if g.KT % 8 == 0 else g.KT     # steady chunk (pairs)
        kc0 = g.KT // 8 if g.KT % 16 == 0 else kc     # first-group chunk
        with (
            tc.tile_pool(name="xt_p", bufs=g.KT // kc0) as xt_p,
            tc.tile_pool(name="wt0_p", bufs=g.KT // kc0) as wt0_p,
            tc.tile_pool(name="wt_p", bufs=3 * (g.KT // kc)) as wt_p,
            tc.tile_pool(name="cand_p", bufs=1) as cand_p,
            tc.tile_pool(name="s_ps", bufs=8, space="PSUM") as s_ps,
        ):
            # x^T and weight slices live in small [P, kchunk, ...] tiles so
            # the first matmul chain only waits on ~0.5MB of DMA
            # (dependencies are tile-granular); group 0 uses half-size
            # chunks to start the PE as early as possible
            def issue_wt(pool, gi, s, kch):
                wts = pool.tile([P, kch, g.GW], FP8, tag="wt",
                                name=f"wt{gi}_{s}")
                nc.sync.dma_start(
                    out=wts[:],
                    in_=wq_d.ap()[:, gi * KGW + s * kch * g.GW:
                                  gi * KGW + (s + 1) * kch * g.GW]
                        .rearrange("p (k w) -> p k w", k=kch))
                return wts

            xts = []

            def issue_xt(s):
                t = xt_p.tile([P, kc0, g.B], FP8, name=f"xt{s}")
                nc.sync.dma_start(
                    out=t[:],
                    in_=xq_d.ap()[:, s * kc0 * g.B:(s + 1) * kc0 * g.B]
                        .rearrange("p (k b) -> p k b", k=kc0))
                xts.append(t)

            # group 0's weights interleaved with x, k-ascending, so the
            # first matmul's inputs (wt0 chunk 0 + xt chunk 0) land first
            wt0 = []
            for s in range(g.KT // kc0):
                wt0.append(issue_wt(wt0_p, 0, s, kc0))
                issue_xt(s)

            candv = cand_p.tile([P, g.CAND], F32)
            candi = cand_p.tile([P, g.CAND], U32)

            for gi in range(g.G):
                if gi == 0:
                    wts, kcg = wt0, kc0
                else:
                    wts = [issue_wt(wt_p, gi, s, kc)
                           for s in range(g.KT // kc)]
                    kcg = kc

                # bi-outer / j-inner: 16-matmul accumulation chains into a
                # single PSUM bank keep the PE at its 216ns/MM stream rate
                # (interleaving banks per-MM measures ~20% slower)
                for bi in range(g.BT):
                    pst = s_ps.tile([P, g.GW], F32, tag="s",
                                    name=f"s{gi}_{bi}")
                    for j in range(g.KP):
                        jj = 2 * j
                        nc.tensor.matmul(
                            pst[:],
                            xts[jj // kc0][:, jj % kc0:jj % kc0 + 2,
                                           bi * P:(bi + 1) * P],
                            wts[jj // kcg][:, jj % kcg:jj % kcg + 2, :],
                            start=(j == 0), stop=(j == g.KP - 1),
                            perf_mode=mybir.MatmulPerfMode.DoubleRow)
                    c0 = gi * B8 + bi * 8
                    nc.vector.max(candv[:, c0:c0 + 8], pst[:])
                    nc.vector.max_index(candi[:, c0:c0 + 8],
                                        candv[:, c0:c0 + 8], pst[:])
                    if gi == g.G - 1:
                        # last group: stream per-bi so only 8 columns of
                        # DMA remain after the final max_index
                        nc.sync.dma_start(out=candv_d.ap()[:, c0:c0 + 8],
                                          in_=candv[:, c0:c0 + 8])
                        nc.sync.dma_start(out=candi_d.ap()[:, c0:c0 + 8],
                                          in_=candi[:, c0:c0 + 8])
                if gi < g.G - 1:
                    # stream this group's candidate slab out immediately
                    nc.sync.dma_start(
                        out=candv_d.ap()[:, gi * B8:(gi + 1) * B8],
                        in_=candv[:, gi * B8:(gi + 1) * B8])
                    nc.sync.dma_start(
                        out=candi_d.ap()[:, gi * B8:(gi + 1) * B8],
                        in_=candi[:, gi * B8:(gi + 1) * B8])

    nc.compile()
    return nc


# --------------------------------------------------------------------------
# host side
# --------------------------------------------------------------------------

def prep_inputs(g: Geom, x, x2y_w, y_age):
    """Normalize + mask + quantize on host; lay out k-major per partition."""
    B, X = g.B, g.X
    xf = np.ascontiguousarray(x.reshape(B, X)).astype(np.float32)
    W = x2y_w
    nrm = np.sqrt(np.einsum("yk,yk->y", W, W))
    mask = (y_age[0] >= 1)
    fac = np.where(mask, SW / np.maximum(nrm, 1e-12), 0.0).astype(np.float32)
    wq8 = (W * fac[:, None]).astype(NP_FP8)               # [Y, X]
    xq8 = (xf * SX).astype(NP_FP8)                        # [B, X]

    # device layouts: [p, k, b] and per-core [p, gi, k, w]
    xq_dev = np.ascontiguousarray(
        xq8.reshape(B, g.KT, P).transpose(2, 1, 0)).reshape(P, g.KT * B)
    wq_dev = np.ascontiguousarray(
        wq8.reshape(g.NC, g.G, g.GW, g.KT, P).transpose(0, 4, 1, 3, 2)
    ).reshape(g.NC, P, g.G * g.KT * g.GW)

    return [{"xq": xq_dev, "wq": wq_dev[c]} for c in range(g.NC)]


def postprocess(g: Geom, results, x, x2y_w, y2z_w, y_age):
    """Resolve global argmax from candidates; exact-rescore margin rows."""
    G8 = g.G * 8
    B = g.B
    NCOL = g.NC * G8
    V = np.empty((B, NCOL), np.float32)
    I = np.empty((B, NCOL), np.int64)
    for c in range(g.NC):
        # device layout: [p, (gi, bi, 8)] with b = bi*128 + p
        cv = np.asarray(results[c]["candv"])                 # [P, CAND]
        ci = np.asarray(results[c]["candi"]).astype(np.int64)
        cv = cv.reshape(P, g.G, g.BT, 8).transpose(2, 0, 1, 3).reshape(B, G8)
        ci = ci.reshape(P, g.G, g.BT, 8).transpose(2, 0, 1, 3).reshape(B, G8)
        V[:, c * G8:(c + 1) * G8] = cv
        I[:, c * G8:(c + 1) * G8] = ci
    base = np.repeat(
        (np.arange(g.NC * g.G, dtype=np.int64) * g.GW), 8)  # [NCOL]
    I += base[None, :]

    xf = x.reshape(B, -1).astype(np.float64)
    xn = np.linalg.norm(xf, axis=1)
    mask = (y_age[0] >= 1)

    def exact_c(b, ys):
        ys = np.asarray(ys, dtype=np.int64)
        Wv = x2y_w[ys, :].astype(np.float64)
        c = (Wv @ xf[b]) / np.linalg.norm(Wv, axis=1) / xn[b]
        return np.where(mask[ys], c, 0.0)

    win = np.empty(B, np.int64)
    tail_rows = []
    n_flagged = n_patched = 0
    max_obs_err = 0.0
    band = 2.0 * DELTA
    for b in range(B):
        vb = V[b] / (SCALE * xn[b])          # x-normalized device scores
        ib = I[b]
        vmax = vb.max()
        win[b] = int(ib[vb == vmax].min())
        in_band = vb >= vmax - band
        if int(in_band.sum()) <= 1:
            continue
        n_flagged += 1
        # guard: a group's 8th (weakest reported) candidate still in band
        # means candidates may be missing -> full exact rescore of the row
        tails = vb.reshape(-1, 8)[:, 7]
        if np.any(tails >= vmax - band):
            tail_rows.append(b)
            continue
        ys, idx = np.unique(ib[in_band], return_index=True)
        ce = exact_c(b, ys)
        max_obs_err = max(max_obs_err,
                          float(np.abs(vb[in_band][idx] - ce).max()))
        w_true = int(ys[np.argmax(ce)])
        if w_true != win[b]:
            n_patched += 1
        win[b] = w_true

    if tail_rows:
        T = len(tail_rows)
        S = np.empty((T, g.Y), np.float64)
        xb = xf[tail_rows]
        CH = 4096
        for y0 in range(0, g.Y, CH):
            Wc = x2y_w[y0:y0 + CH].astype(np.float64)
            S[:, y0:y0 + CH] = (xb @ Wc.T) / \
                np.linalg.norm(Wc, axis=1)[None, :]
        S[:, ~mask] = 0.0
        for t, b in enumerate(tail_rows):
            w_true = int(np.argmax(S[t]))
            if w_true != win[b]:
                n_patched += 1
            win[b] = w_true

    n2 = np.sqrt(np.einsum("zy,zy->z", y2z_w, y2z_w, dtype=np.float64))
    n2 = np.maximum(n2, 1e-12)
    out = (y2z_w[:, win].astype(np.float64) / n2[:, None]).T
    postprocess.stats = {"flagged": n_flagged, "patched": n_patched,
                         "full_rescore": len(tail_rows),
                         "max_obs_err": max_obs_err}
    return np.ascontiguousarray(out.astype(np.float32))


_BUILT = {}


def _get_nc(g: Geom):
    if g not in _BUILT:
        _BUILT[g] = build_nc(g)
    return _BUILT[g]


def kernel(**inputs) -> np.ndarray:
    global LAST_RESULTS
    g = FULL
    x = np.asarray(inputs["x"], dtype=np.float32)
    x2y_w = np.asarray(inputs["x2y_w"], dtype=np.float32)
    y2z_w = np.asarray(inputs["y2z_w"], dtype=np.float32)
    y_age = np.asarray(inputs["y_age"])

    nc = _get_nc(g)
    in_maps = prep_inputs(g, x, x2y_w, y_age)
    res = run_bass_kernel_spmd(nc, in_maps, list(range(g.NC)),
                               trace=TRACE, **TRACE_KWARGS)
    LAST_RESULTS = res
    return postprocess(g, res.results, x, x2y_w, y2z_w, y_age)
